# revision 12
# baseline (speedup 1.0000x reference)
"""DeepSeek-V3-style MoE layer on 8 Trainium2 NeuronCores.

Strategy (expert-parallel, host-routed):
  - Gate (sigmoid + group-limited top-k) is computed on host with jax/CPU,
    mirroring the reference ops exactly so expert selection is bit-identical.
  - Tokens are gathered per expert on host, transposed to [DIM, C] so the
    device kernel is a pure grouped GEMM:
        hT = silu(W1 @ xgT) * (W3 @ xgT);  ygT = W2 @ hT
  - Experts are ranked by token count and placed into 4 "slots" per core
    (rank group g -> slot g); slot capacity = max count in its rank group.
    All 8 cores run the identical program (SPMD) with per-slot capacities
    baked in -> perfect load balance, ~1.5% padding.
  - All device math in bf16 (PE runs 1 col/cycle for bf16 and fp32; bf16
    halves DMA and enables fast weight load; fp8 fails the accuracy gate).
    PSUM accumulates fp32. Matmul moving dim is 512 (PSUM bank limit);
    remainder chunks are narrow matmuls which are cheap in bf16.
  - Weights are DMA'd ONCE per expert (token-chunk loop is inside the
    weight-tile loop).
  - DMA issue on the Sync queue costs ~0.6us each, so transfers are
    batched: one DMA per 512-token x chunk (host pre-packs [128, KT*cw]
    blocks), w1+w3 fused per inter-tile, w2 in pairs, and one output DMA
    per of the 16 output row-tiles per expert.
  - Each core also runs a 1/8 token-slice of the shared SwiGLU expert.
  - Host applies routed combine weights during the scatter-add epilogue.
"""

import numpy as np
import ml_dtypes

DIM = 2048
INTER = 1408
N_EXPERTS = 32
TOPK = 6
N_GROUPS = 8
TOPK_GROUPS = 4
ROUTE_SCALE = 2.5
SHARED_INTER = 2816
T = 8192

NCORES = 8
ELOC = N_EXPERTS // NCORES          # 4 expert slots per core
TS = T // NCORES                    # 1024 shared-expert tokens per core
KT = DIM // 128                     # 16 contraction tiles (dim)
MT = INTER // 128                   # 11 inter tiles
SMT = SHARED_INTER // 128           # 22 shared inter tiles

BF16 = ml_dtypes.bfloat16

_prog_cache = {}


def _gate_host(x, gate_w):
    """Bit-identical copy of the reference gate, forced onto jax CPU."""
    import jax
    import jax.numpy as jnp

    cpu = jax.devices("cpu")[0]
    with jax.default_device(cpu):
        xj = jnp.asarray(x)
        gj = jnp.asarray(gate_w)
        scores = jax.nn.sigmoid(xj @ gj.T)
        original = scores
        sg = scores.reshape(x.shape[0], N_GROUPS, -1)
        group_scores = sg.max(axis=-1)
        _, gidx = jax.lax.top_k(group_scores, TOPK_GROUPS)
        gmask = jnp.zeros((x.shape[0], N_GROUPS), bool).at[
            jnp.arange(x.shape[0])[:, None], gidx].set(True)
        masked = jnp.where(gmask[:, :, None], sg, 0.0).reshape(x.shape[0], -1)
        _, idx = jax.lax.top_k(masked, TOPK)
        w = jnp.take_along_axis(original, idx, axis=1)
        w = w / w.sum(axis=-1, keepdims=True)
        w = w * ROUTE_SCALE
        return np.asarray(w, dtype=np.float32), np.asarray(idx, dtype=np.int32)


def _chunks(width):
    """Token-chunk (off, w) list: full 512s plus one remainder."""
    out = []
    off = 0
    while off < width:
        w = min(512, width - off)
        out.append((off, w))
        off += w
    return out


def _build_program(slot_caps):
    import concourse.tile as tile
    from concourse import bacc, mybir

    f32 = mybir.dt.float32
    bf = mybir.dt.bfloat16
    AF = mybir.ActivationFunctionType

    nc = bacc.Bacc(None, target_bir_lowering=False)

    # x chunks are packed [128, KT*cw] per chunk (k-blocks side by side)
    xg_d = [nc.dram_tensor(f"xg{j}", [128, KT * s], bf, kind="ExternalInput")
            for j, s in enumerate(slot_caps)]
    w13_d = nc.dram_tensor("w13t", [ELOC, MT, 128, 2 * DIM], bf,
                           kind="ExternalInput")
    w2_d = nc.dram_tensor("w2t", [ELOC, KT // 2, 128, 2 * INTER], bf,
                          kind="ExternalInput")
    xs_d = nc.dram_tensor("xs", [128, KT * TS], bf, kind="ExternalInput")
    sw13_d = nc.dram_tensor("sw13t", [SMT, 128, 2 * DIM], bf,
                            kind="ExternalInput")
    sw2_d = nc.dram_tensor("sw2t", [KT // 2, 128, 2 * SHARED_INTER], bf,
                           kind="ExternalInput")
    yg_d = [nc.dram_tensor(f"yg{j}", [KT, 128, s], bf, kind="ExternalOutput")
            for j, s in enumerate(slot_caps)]
    zs_d = nc.dram_tensor("zs", [KT, 128, TS], bf, kind="ExternalOutput")

    with tile.TileContext(nc) as tc:
        with tc.tile_pool(name="main", bufs=1) as mp, \
             tc.tile_pool(name="psum", bufs=1, space="PSUM") as pp:

            def load_x(x_src, chunk_list, nm):
                """One packed [128, KT*cw] tile + one DMA per token chunk.

                x DMAs ride the (otherwise idle) GpSimd queue so the next
                expert's first chunk streams in during the current expert's
                compute instead of queueing behind output DMAs on Sync."""
                xc = []
                for ci, (off, cw) in enumerate(chunk_list):
                    xt = mp.tile([128, KT * cw], bf, tag="xg", bufs=5,
                                 name=f"{nm}_{ci}")
                    if ci == 0:
                        # split the first chunk across two queues so the
                        # first matmul group's operands land ~2x sooner
                        half = (KT // 2) * cw
                        nc.sync.dma_start(
                            out=xt[:, :half],
                            in_=x_src[:, KT * off:KT * off + half])
                        nc.gpsimd.dma_start(
                            out=xt[:, half:],
                            in_=x_src[:, KT * off + half:KT * (off + cw)])
                    else:
                        nc.gpsimd.dma_start(
                            out=xt, in_=x_src[:, KT * off:KT * (off + cw)])
                    xc.append(xt)
                return xc

            def mlp(xc, n_mt, w13_src, w2_src, chunk_list, y_sink,
                    w13_pre=None):
                """One SwiGLU MLP over packed token-chunk tiles in SBUF.

                xc[ci]: [128, KT*cw] bf16, k-block k at cols [k*cw,(k+1)*cw).
                w13_src[m] -> [128, 2*DIM] (w1 tile | w3 tile).
                w2_src[q] -> [128, 2*n_mt*128] (m2=2q | m2=2q+1).
                """
                h_tiles = [[None] * len(chunk_list) for _ in range(n_mt)]
                for m in range(n_mt):
                    if m == 0 and w13_pre is not None:
                        w13_t = w13_pre
                    else:
                        w13_t = mp.tile([128, 2 * DIM], bf, tag="w13", bufs=3,
                                        name=f"w13_{m}")
                        nc.sync.dma_start(out=w13_t, in_=w13_src[m])
                    for ci, (off, cw) in enumerate(chunk_list):
                        pa = pp.tile([128, cw], f32, tag="pa", bufs=3, name="pa")
                        pb = pp.tile([128, cw], f32, tag="pb", bufs=3, name="pb")
                        for k in range(KT):
                            nc.tensor.matmul(
                                pa, lhsT=w13_t[:, k * 128:(k + 1) * 128],
                                rhs=xc[ci][:, k * cw:(k + 1) * cw],
                                start=(k == 0), stop=(k == KT - 1))
                        for k in range(KT):
                            nc.tensor.matmul(
                                pb, lhsT=w13_t[:, DIM + k * 128:DIM + (k + 1) * 128],
                                rhs=xc[ci][:, k * cw:(k + 1) * cw],
                                start=(k == 0), stop=(k == KT - 1))
                        sil = mp.tile([128, cw], bf, tag="sil", bufs=3, name="sil")
                        nc.scalar.activation(out=sil, in_=pa, func=AF.Silu)
                        h_t = mp.tile([128, cw], bf, tag="h", bufs=44,
                                      name=f"h_{m}_{ci}")
                        nc.vector.tensor_mul(h_t, sil, pb)
                        h_tiles[m][ci] = h_t
                W = chunk_list[-1][0] + chunk_list[-1][1]
                for q in range(KT // 2):
                    w2_t = mp.tile([128, 2 * n_mt * 128], bf,
                                   tag=("w2" if n_mt == MT else "sw2"), bufs=2,
                                   name=f"w2_{q}")
                    nc.sync.dma_start(out=w2_t, in_=w2_src[q])
                    for sub in range(2):
                        m2 = 2 * q + sub
                        base = sub * n_mt * 128
                        yo = mp.tile([128, W], bf, tag="yo", bufs=3,
                                     name=f"yo_{m2}")
                        for ci, (off, cw) in enumerate(chunk_list):
                            py = pp.tile([128, cw], f32, tag="py", bufs=2,
                                         name="py")
                            for k2 in range(n_mt):
                                nc.tensor.matmul(
                                    py,
                                    lhsT=w2_t[:, base + k2 * 128:base + (k2 + 1) * 128],
                                    rhs=h_tiles[k2][ci],
                                    start=(k2 == 0), stop=(k2 == n_mt - 1))
                            nc.vector.tensor_copy(yo[:, off:off + cw], py)
                        y_sink(m2, yo)

            # ---- routed expert slots ----
            for j, s in enumerate(slot_caps):
                chunk_list = _chunks(s)
                w13_pre = None
                if j == 0:
                    # first weight tile ahead of the x stream: the first
                    # matmul needs both, and the SDMA rings drain in
                    # doorbell order
                    w13_pre = mp.tile([128, 2 * DIM], bf, tag="w13", bufs=3,
                                      name="w13_pre")
                    nc.sync.dma_start(out=w13_pre, in_=w13_d[j][0])
                xc = load_x(xg_d[j], chunk_list, f"xg{j}")

                def y_sink(m2, yo, j=j):
                    # outputs ride the Scalar queue: keeps the Sync queue a
                    # pure weight-prefetch stream with no head-of-line block
                    nc.scalar.dma_start(out=yg_d[j][m2], in_=yo)

                mlp(xc, MT, w13_d[j], w2_d[j], chunk_list, y_sink,
                    w13_pre=w13_pre)

            # ---- shared expert (1/8 token slice) ----
            chunk_list = _chunks(TS)
            xc = load_x(xs_d, chunk_list, "xs")

            def z_sink(m2, yo):
                nc.scalar.dma_start(out=zs_d[m2], in_=yo)

            mlp(xc, SMT, sw13_d, sw2_d, chunk_list, z_sink)

    nc.finalize()
    return nc


def _get_program(slot_caps):
    key = tuple(slot_caps)
    if key not in _prog_cache:
        _prog_cache[key] = _build_program(key)
    return _prog_cache[key]


def _wtiles(w):
    """[out, in] -> [out/128, 128(in-part), in/128*128(out-col)] bf16 so each
    per-m weight block works as a run of [128,128] lhsT tiles."""
    o, i = w.shape
    t = w.reshape(o // 128, 128, i // 128, 128).transpose(0, 3, 2, 1)
    return np.ascontiguousarray(t.reshape(o // 128, 128, i).astype(BF16))


def _pack_pairs(wt):
    """[KT, 128, I] -> [KT/2, 128, 2*I] (adjacent m2 tiles side by side)."""
    n, p, i = wt.shape
    return np.ascontiguousarray(
        wt.reshape(n // 2, 2, p, i).transpose(0, 2, 1, 3).reshape(n // 2, p, 2 * i))


def _pack_x_chunks(xTb, toks, s):
    """Gathered tokens -> [128, KT*s] with per-chunk [128, KT*cw] k-blocks."""
    out = np.zeros((128, KT * s), dtype=BF16)
    n = len(toks)
    for off, cw in _chunks(s):
        m = max(0, min(cw, n - off))
        if m == 0:
            break
        blk = np.zeros((DIM, cw), dtype=BF16)
        np.take(xTb, toks[off:off + m], axis=1, out=blk[:, :m])
        out[:, KT * off:KT * (off + cw)] = \
            blk.reshape(KT, 128, cw).transpose(1, 0, 2).reshape(128, KT * cw)
    return out


def kernel(x, gate_w, w1, w2, w3, sw1, sw2, sw3):
    from concourse.bass_utils import run_bass_kernel_spmd

    x = np.ascontiguousarray(np.asarray(x, dtype=np.float32))
    gate_w = np.asarray(gate_w, dtype=np.float32)
    w1 = np.asarray(w1, dtype=np.float32)
    w2 = np.asarray(w2, dtype=np.float32)
    w3 = np.asarray(w3, dtype=np.float32)
    sw1 = np.asarray(sw1, dtype=np.float32)
    sw2 = np.asarray(sw2, dtype=np.float32)
    sw3 = np.asarray(sw3, dtype=np.float32)

    # ---- host routing (bit-identical to reference gate) ----
    weights, idx = _gate_host(x, gate_w)

    flat_e = idx.ravel()
    flat_tok = np.repeat(np.arange(T, dtype=np.int64), TOPK)
    flat_w = weights.ravel()
    order = np.argsort(flat_e, kind="stable")
    sorted_tok = flat_tok[order]
    sorted_w = flat_w[order]
    counts = np.bincount(flat_e, minlength=N_EXPERTS)
    offs = np.concatenate([[0], np.cumsum(counts)])

    # ---- slot assignment: rank experts by count, group by rank octile ----
    rank = np.argsort(-counts, kind="stable")        # experts, largest first
    # slot j on core c handles expert rank[j*8 + c]
    slot_caps = []
    for j in range(ELOC):
        grp = counts[rank[j * 8:(j + 1) * 8]]
        cap = max(128, int(-(-int(grp.max()) // 8)) * 8)
        slot_caps.append(cap)
    slot_caps = tuple(slot_caps)

    # ---- host data prep (bf16 layouts) ----
    xTb = np.ascontiguousarray(x.T.astype(BF16))     # [DIM, T] bf16

    nc = _get_program(slot_caps)

    sw13 = np.concatenate([_wtiles(sw1), _wtiles(sw3)], axis=-1)
    sw2p = _pack_pairs(_wtiles(sw2))

    in_maps = []
    for core in range(NCORES):
        m = {}
        es = [int(rank[j * 8 + core]) for j in range(ELOC)]
        for j, e in enumerate(es):
            te = sorted_tok[offs[e]:offs[e + 1]]
            m[f"xg{j}"] = _pack_x_chunks(xTb, te, slot_caps[j])
        m["w13t"] = np.stack(
            [np.concatenate([_wtiles(w1[e]), _wtiles(w3[e])], axis=-1)
             for e in es])
        m["w2t"] = np.stack([_pack_pairs(_wtiles(w2[e])) for e in es])
        m["xs"] = _pack_x_chunks(
            xTb, np.arange(core * TS, (core + 1) * TS), TS)
        m["sw13t"] = sw13
        m["sw2t"] = sw2p
        in_maps.append(m)

    res = run_bass_kernel_spmd(nc, in_maps, core_ids=list(range(NCORES)))

    # ---- host epilogue: combine-weight scatter-add + shared add ----
    y = np.zeros((T, DIM), dtype=np.float32)
    for core in range(NCORES):
        r = res.results[core]
        for j in range(ELOC):
            e = int(rank[j * 8 + core])
            cnt = int(counts[e])
            if cnt == 0:
                continue
            toks = sorted_tok[offs[e]:offs[e + 1]]
            cw = sorted_w[offs[e]:offs[e + 1]]
            yg = r[f"yg{j}"].reshape(DIM, slot_caps[j])[:, :cnt]
            # toks are unique within one expert (top-k indices are distinct)
            y[toks] += cw[:, None] * yg.T.astype(np.float32)
        y[core * TS:(core + 1) * TS] += \
            r["zs"].reshape(DIM, TS).T.astype(np.float32)
    return y


# revision 15
# speedup vs baseline: 1.0003x; 1.0003x over previous
"""DeepSeek-V3-style MoE layer on 8 Trainium2 NeuronCores.

Strategy (expert-parallel, host-routed):
  - Gate (sigmoid + group-limited top-k) is computed on host with jax/CPU,
    mirroring the reference ops exactly so expert selection is bit-identical.
  - Tokens are gathered per expert on host, transposed to [DIM, C] so the
    device kernel is a pure grouped GEMM:
        hT = silu(W1 @ xgT) * (W3 @ xgT);  ygT = W2 @ hT
  - Experts are ranked by token count and placed into 4 "slots" per core
    (rank group g -> slot g); slot capacity = max count in its rank group.
    All 8 cores run the identical program (SPMD) with per-slot capacities
    baked in -> perfect load balance, ~1.5% padding.
  - All device math in bf16 (PE runs 1 col/cycle for bf16 and fp32; bf16
    halves DMA and enables fast weight load; fp8 fails the accuracy gate).
    PSUM accumulates fp32. Matmul moving dim is 512 (PSUM bank limit);
    remainder chunks are narrow matmuls which are cheap in bf16.
  - Weights are DMA'd ONCE per expert (token-chunk loop is inside the
    weight-tile loop).
  - DMA issue on the Sync queue costs ~0.6us each, so transfers are
    batched: one DMA per 512-token x chunk (host pre-packs [128, KT*cw]
    blocks), w1+w3 fused per inter-tile, w2 in pairs, and one output DMA
    per of the 16 output row-tiles per expert.
  - Each core also runs a 1/8 token-slice of the shared SwiGLU expert.
  - Host applies routed combine weights during the scatter-add epilogue.
"""

import numpy as np
import ml_dtypes

DIM = 2048
INTER = 1408
N_EXPERTS = 32
TOPK = 6
N_GROUPS = 8
TOPK_GROUPS = 4
ROUTE_SCALE = 2.5
SHARED_INTER = 2816
T = 8192

NCORES = 8
ELOC = N_EXPERTS // NCORES          # 4 expert slots per core
TS = T // NCORES                    # 1024 shared-expert tokens per core
KT = DIM // 128                     # 16 contraction tiles (dim)
MT = INTER // 128                   # 11 inter tiles
SMT = SHARED_INTER // 128           # 22 shared inter tiles

BF16 = ml_dtypes.bfloat16

_prog_cache = {}


def _gate_host(x, gate_w):
    """Bit-identical copy of the reference gate, forced onto jax CPU."""
    import jax
    import jax.numpy as jnp

    cpu = jax.devices("cpu")[0]
    with jax.default_device(cpu):
        xj = jnp.asarray(x)
        gj = jnp.asarray(gate_w)
        scores = jax.nn.sigmoid(xj @ gj.T)
        original = scores
        sg = scores.reshape(x.shape[0], N_GROUPS, -1)
        group_scores = sg.max(axis=-1)
        _, gidx = jax.lax.top_k(group_scores, TOPK_GROUPS)
        gmask = jnp.zeros((x.shape[0], N_GROUPS), bool).at[
            jnp.arange(x.shape[0])[:, None], gidx].set(True)
        masked = jnp.where(gmask[:, :, None], sg, 0.0).reshape(x.shape[0], -1)
        _, idx = jax.lax.top_k(masked, TOPK)
        w = jnp.take_along_axis(original, idx, axis=1)
        w = w / w.sum(axis=-1, keepdims=True)
        w = w * ROUTE_SCALE
        return np.asarray(w, dtype=np.float32), np.asarray(idx, dtype=np.int32)


def _chunks(width):
    """Token-chunk (off, w) list: full 512s plus one remainder."""
    out = []
    off = 0
    while off < width:
        w = min(512, width - off)
        out.append((off, w))
        off += w
    return out


def _build_program(slot_caps):
    import concourse.tile as tile
    from concourse import bacc, mybir

    f32 = mybir.dt.float32
    bf = mybir.dt.bfloat16
    AF = mybir.ActivationFunctionType

    nc = bacc.Bacc(None, target_bir_lowering=False)

    # x chunks are packed [128, KT*cw] per chunk (k-blocks side by side)
    xg_d = [nc.dram_tensor(f"xg{j}", [128, KT * s], bf, kind="ExternalInput")
            for j, s in enumerate(slot_caps)]
    w13_d = nc.dram_tensor("w13t", [ELOC, MT, 128, 2 * DIM], bf,
                           kind="ExternalInput")
    w2_d = nc.dram_tensor("w2t", [ELOC, KT // 2, 128, 2 * INTER], bf,
                          kind="ExternalInput")
    xs_d = nc.dram_tensor("xs", [128, KT * TS], bf, kind="ExternalInput")
    sw13_d = nc.dram_tensor("sw13t", [SMT, 128, 2 * DIM], bf,
                            kind="ExternalInput")
    sw2_d = nc.dram_tensor("sw2t", [KT // 2, 128, 2 * SHARED_INTER], bf,
                           kind="ExternalInput")
    yg_d = [nc.dram_tensor(f"yg{j}", [KT, 128, s], bf, kind="ExternalOutput")
            for j, s in enumerate(slot_caps)]
    zs_d = nc.dram_tensor("zs", [KT, 128, TS], bf, kind="ExternalOutput")

    with tile.TileContext(nc) as tc:
        with tc.tile_pool(name="main", bufs=1) as mp, \
             tc.tile_pool(name="psum", bufs=1, space="PSUM") as pp:

            def load_x(x_src, chunk_list, nm):
                """One packed [128, KT*cw] tile + one DMA per token chunk.

                x DMAs ride the (otherwise idle) GpSimd queue so the next
                expert's first chunk streams in during the current expert's
                compute instead of queueing behind output DMAs on Sync."""
                xc = []
                for ci, (off, cw) in enumerate(chunk_list):
                    xt = mp.tile([128, KT * cw], bf, tag="xg", bufs=5,
                                 name=f"{nm}_{ci}")
                    if ci == 0:
                        # split the first chunk across two queues so the
                        # first matmul group's operands land ~2x sooner
                        half = (KT // 2) * cw
                        nc.sync.dma_start(
                            out=xt[:, :half],
                            in_=x_src[:, KT * off:KT * off + half])
                        nc.gpsimd.dma_start(
                            out=xt[:, half:],
                            in_=x_src[:, KT * off + half:KT * (off + cw)])
                    else:
                        nc.gpsimd.dma_start(
                            out=xt, in_=x_src[:, KT * off:KT * (off + cw)])
                    xc.append(xt)
                return xc

            def mlp(xc, n_mt, w13_src, w2_src, chunk_list, y_sink):
                """One SwiGLU MLP over packed token-chunk tiles in SBUF.

                xc[ci]: [128, KT*cw] bf16, k-block k at cols [k*cw,(k+1)*cw).
                w13_src[m] -> [128, 2*DIM] (w1 tile | w3 tile).
                w2_src[q] -> [128, 2*n_mt*128] (m2=2q | m2=2q+1).
                """
                h_tiles = [[None] * len(chunk_list) for _ in range(n_mt)]
                for m in range(n_mt):
                    w13_t = mp.tile([128, 2 * DIM], bf, tag="w13", bufs=3,
                                    name=f"w13_{m}")
                    nc.sync.dma_start(out=w13_t, in_=w13_src[m])
                    for ci, (off, cw) in enumerate(chunk_list):
                        pa = pp.tile([128, cw], f32, tag="pa", bufs=3, name="pa")
                        pb = pp.tile([128, cw], f32, tag="pb", bufs=3, name="pb")
                        for k in range(KT):
                            nc.tensor.matmul(
                                pa, lhsT=w13_t[:, k * 128:(k + 1) * 128],
                                rhs=xc[ci][:, k * cw:(k + 1) * cw],
                                start=(k == 0), stop=(k == KT - 1))
                        for k in range(KT):
                            nc.tensor.matmul(
                                pb, lhsT=w13_t[:, DIM + k * 128:DIM + (k + 1) * 128],
                                rhs=xc[ci][:, k * cw:(k + 1) * cw],
                                start=(k == 0), stop=(k == KT - 1))
                        sil = mp.tile([128, cw], bf, tag="sil", bufs=3, name="sil")
                        nc.scalar.activation(out=sil, in_=pa, func=AF.Silu)
                        h_t = mp.tile([128, cw], bf, tag="h", bufs=44,
                                      name=f"h_{m}_{ci}")
                        nc.vector.tensor_mul(h_t, sil, pb)
                        h_tiles[m][ci] = h_t
                W = chunk_list[-1][0] + chunk_list[-1][1]
                for q in range(KT // 2):
                    w2_t = mp.tile([128, 2 * n_mt * 128], bf,
                                   tag=("w2" if n_mt == MT else "sw2"), bufs=2,
                                   name=f"w2_{q}")
                    nc.sync.dma_start(out=w2_t, in_=w2_src[q])
                    for sub in range(2):
                        m2 = 2 * q + sub
                        base = sub * n_mt * 128
                        yo = mp.tile([128, W], bf, tag="yo", bufs=3,
                                     name=f"yo_{m2}")
                        for ci, (off, cw) in enumerate(chunk_list):
                            py = pp.tile([128, cw], f32, tag="py", bufs=2,
                                         name="py")
                            for k2 in range(n_mt):
                                nc.tensor.matmul(
                                    py,
                                    lhsT=w2_t[:, base + k2 * 128:base + (k2 + 1) * 128],
                                    rhs=h_tiles[k2][ci],
                                    start=(k2 == 0), stop=(k2 == n_mt - 1))
                            nc.vector.tensor_copy(yo[:, off:off + cw], py)
                        y_sink(m2, yo)

            # ---- routed expert slots ----
            for j, s in enumerate(slot_caps):
                chunk_list = _chunks(s)
                xc = load_x(xg_d[j], chunk_list, f"xg{j}")

                def y_sink(m2, yo, j=j):
                    # outputs ride the Scalar queue: keeps the Sync queue a
                    # pure weight-prefetch stream with no head-of-line block
                    nc.scalar.dma_start(out=yg_d[j][m2], in_=yo)

                mlp(xc, MT, w13_d[j], w2_d[j], chunk_list, y_sink)

            # ---- shared expert (1/8 token slice) ----
            chunk_list = _chunks(TS)
            xc = load_x(xs_d, chunk_list, "xs")

            def z_sink(m2, yo):
                nc.scalar.dma_start(out=zs_d[m2], in_=yo)

            mlp(xc, SMT, sw13_d, sw2_d, chunk_list, z_sink)

    nc.finalize()
    return nc


def _get_program(slot_caps):
    key = tuple(slot_caps)
    if key not in _prog_cache:
        _prog_cache[key] = _build_program(key)
    return _prog_cache[key]


def _wtiles(w):
    """[out, in] -> [out/128, 128(in-part), in/128*128(out-col)] bf16 so each
    per-m weight block works as a run of [128,128] lhsT tiles."""
    o, i = w.shape
    t = w.reshape(o // 128, 128, i // 128, 128).transpose(0, 3, 2, 1)
    return np.ascontiguousarray(t.reshape(o // 128, 128, i).astype(BF16))


def _pack_pairs(wt):
    """[KT, 128, I] -> [KT/2, 128, 2*I] (adjacent m2 tiles side by side)."""
    n, p, i = wt.shape
    return np.ascontiguousarray(
        wt.reshape(n // 2, 2, p, i).transpose(0, 2, 1, 3).reshape(n // 2, p, 2 * i))


def _pack_x_chunks(xTb, toks, s):
    """Gathered tokens -> [128, KT*s] with per-chunk [128, KT*cw] k-blocks."""
    out = np.zeros((128, KT * s), dtype=BF16)
    n = len(toks)
    for off, cw in _chunks(s):
        m = max(0, min(cw, n - off))
        if m == 0:
            break
        blk = np.zeros((DIM, cw), dtype=BF16)
        np.take(xTb, toks[off:off + m], axis=1, out=blk[:, :m])
        out[:, KT * off:KT * (off + cw)] = \
            blk.reshape(KT, 128, cw).transpose(1, 0, 2).reshape(128, KT * cw)
    return out


def kernel(x, gate_w, w1, w2, w3, sw1, sw2, sw3):
    from concourse.bass_utils import run_bass_kernel_spmd

    x = np.ascontiguousarray(np.asarray(x, dtype=np.float32))
    gate_w = np.asarray(gate_w, dtype=np.float32)
    w1 = np.asarray(w1, dtype=np.float32)
    w2 = np.asarray(w2, dtype=np.float32)
    w3 = np.asarray(w3, dtype=np.float32)
    sw1 = np.asarray(sw1, dtype=np.float32)
    sw2 = np.asarray(sw2, dtype=np.float32)
    sw3 = np.asarray(sw3, dtype=np.float32)

    # ---- host routing (bit-identical to reference gate) ----
    weights, idx = _gate_host(x, gate_w)

    flat_e = idx.ravel()
    flat_tok = np.repeat(np.arange(T, dtype=np.int64), TOPK)
    flat_w = weights.ravel()
    order = np.argsort(flat_e, kind="stable")
    sorted_tok = flat_tok[order]
    sorted_w = flat_w[order]
    counts = np.bincount(flat_e, minlength=N_EXPERTS)
    offs = np.concatenate([[0], np.cumsum(counts)])

    # ---- slot assignment: rank experts by count, group by rank octile ----
    rank = np.argsort(-counts, kind="stable")        # experts, largest first
    # slot j on core c handles expert rank[j*8 + c]
    slot_caps = []
    for j in range(ELOC):
        grp = counts[rank[j * 8:(j + 1) * 8]]
        cap = max(128, int(-(-int(grp.max()) // 8)) * 8)
        slot_caps.append(cap)
    slot_caps = tuple(slot_caps)

    # ---- host data prep (bf16 layouts) ----
    xTb = np.ascontiguousarray(x.T.astype(BF16))     # [DIM, T] bf16

    nc = _get_program(slot_caps)

    sw13 = np.concatenate([_wtiles(sw1), _wtiles(sw3)], axis=-1)
    sw2p = _pack_pairs(_wtiles(sw2))

    in_maps = []
    for core in range(NCORES):
        m = {}
        es = [int(rank[j * 8 + core]) for j in range(ELOC)]
        for j, e in enumerate(es):
            te = sorted_tok[offs[e]:offs[e + 1]]
            m[f"xg{j}"] = _pack_x_chunks(xTb, te, slot_caps[j])
        m["w13t"] = np.stack(
            [np.concatenate([_wtiles(w1[e]), _wtiles(w3[e])], axis=-1)
             for e in es])
        m["w2t"] = np.stack([_pack_pairs(_wtiles(w2[e])) for e in es])
        m["xs"] = _pack_x_chunks(
            xTb, np.arange(core * TS, (core + 1) * TS), TS)
        m["sw13t"] = sw13
        m["sw2t"] = sw2p
        in_maps.append(m)

    res = run_bass_kernel_spmd(nc, in_maps, core_ids=list(range(NCORES)))

    # ---- host epilogue: combine-weight scatter-add + shared add ----
    y = np.zeros((T, DIM), dtype=np.float32)
    for core in range(NCORES):
        r = res.results[core]
        for j in range(ELOC):
            e = int(rank[j * 8 + core])
            cnt = int(counts[e])
            if cnt == 0:
                continue
            toks = sorted_tok[offs[e]:offs[e + 1]]
            cw = sorted_w[offs[e]:offs[e + 1]]
            yg = r[f"yg{j}"].reshape(DIM, slot_caps[j])[:, :cnt]
            # toks are unique within one expert (top-k indices are distinct)
            y[toks] += cw[:, None] * yg.T.astype(np.float32)
        y[core * TS:(core + 1) * TS] += \
            r["zs"].reshape(DIM, TS).T.astype(np.float32)
    return y
